# revision 4
# baseline (speedup 1.0000x reference)
"""Trainium2 Bass kernel for nn_LinearWaypointsPredictor.

Reference computation (dense 6-expert MLP + per-sample expert select + cumsum):
    xr = x + rank_embed                                   # [bs, 10, 256]
    h  = relu(einsum('bnd,edh->benh', xr, W1) + b1)       # [bs, 6, 10, 64]
    wp = einsum('benh,eho->beno', h, W2) + b2             # [bs, 6, 10, 2]
    sel = wp[b, measurements[b]]                          # [bs, 10, 2]
    out = cumsum(sel, axis=1)

Strategy (pure data parallel over 8 cores, batch-sharded):
  - rank_embed is folded on the host into per-waypoint layer-1 biases:
      b1_eff[n,e,:] = b1[e] + rank_embed[0,n,:] @ W1[e]
    so the device never touches rank_embed or the elementwise add.
  - Compute in fp16 (PE runs fp16 at 1 cycle/row vs 4 for fp32; ~1e-3 rel).
  - Per core: loop over 8 column blocks (512 samples) x 10 waypoints.
    Columns of every on-chip tile are samples; x tiles are loaded directly
    transposed ([d, sample]) by the xbar DMA-transpose path (2-byte dtype).
  - Layer 1: 6 weight-stationary matmuls (3 expert pairs x 2 d-chunks)
    accumulating into 3 PSUM tiles [128, 512]; relu+bias on ACT/DVE.
  - Layer 2: 3 block-diagonal matmuls (K=128, M=32, col-tiled) -> wp [96,512].
  - Selection: mask[r, col] = (meas[col] == expert(r)) built once per column
    block; masked = (wp + b2) * mask.
  - Cumsum + selection reduce: one accumulating matmul per waypoint with a
    constant selector lhsT [96, 20]: selmat_n[r, 2q+o] = 1{q>=n, r%2==o,
    r active}; PSUM accumulates over n giving the full [10,2] prefix sums.
  - Output [20, 512] is PE-transposed to [128, 20] tiles and DMA'd out.
"""

import numpy as np

import concourse.bass as bass
import concourse.tile as tile
from concourse import bacc, mybir
from concourse.bass_utils import run_bass_kernel_spmd
from concourse.masks import make_identity

BS, NWP, D, NEXP, H, OUT = 32768, 10, 256, 6, 64, 2
N_CORES = 8
BSH = BS // N_CORES          # samples per core (4096)
CBS = 512                    # samples per column block
NCB = BSH // CBS             # column blocks per core (8)
NPAIR = 3                    # expert pairs
NK = D // 128                # d chunks (2)

F16 = mybir.dt.float16
F32 = mybir.dt.float32


def _build_program(reps: int = 1):
    nc = bacc.Bacc("TRN2", target_bir_lowering=False, debug=False,
                   num_devices=N_CORES)

    x_t = nc.dram_tensor("x16", [BSH, NWP, D], F16, kind="ExternalInput")
    meas_t = nc.dram_tensor("measf", [BSH], F16, kind="ExternalInput")
    w1_t = nc.dram_tensor("w1p", [NPAIR, NK, 128, 128], F16, kind="ExternalInput")
    w2_t = nc.dram_tensor("w2b", [NPAIR, 128, 32], F16, kind="ExternalInput")
    b1_t = nc.dram_tensor("b1e", [NWP, NPAIR, 128], F32, kind="ExternalInput")
    b2_t = nc.dram_tensor("b2r", [96, 1], F32, kind="ExternalInput")
    er_t = nc.dram_tensor("erow", [96, 1], F32, kind="ExternalInput")
    sel_t = nc.dram_tensor("selm", [NWP, 96, 20], F16, kind="ExternalInput")
    out_t = nc.dram_tensor("out", [BSH, NWP, OUT], F32, kind="ExternalOutput")

    xap = x_t.ap()
    outap = out_t.ap().rearrange("b n o -> b (n o)")

    from contextlib import ExitStack

    with tile.TileContext(nc) as tc, ExitStack() as ctx:
        consts = ctx.enter_context(tc.tile_pool(name="consts", bufs=1))
        xin = ctx.enter_context(tc.tile_pool(name="xin", bufs=4))
        hpool = ctx.enter_context(tc.tile_pool(name="hpool", bufs=6))
        wk = ctx.enter_context(tc.tile_pool(name="wk", bufs=2))
        opool = ctx.enter_context(tc.tile_pool(name="opool", bufs=2))
        psH = ctx.enter_context(tc.tile_pool(name="psH", bufs=4, space="PSUM"))
        ps96 = ctx.enter_context(tc.tile_pool(name="ps96", bufs=2, space="PSUM"))
        ps20 = ctx.enter_context(tc.tile_pool(name="ps20", bufs=1, space="PSUM"))
        pstp = ctx.enter_context(tc.tile_pool(name="pstp", bufs=1, space="PSUM"))

        if True:
            # --- constants, loaded once ---
            w1sb = []
            for p in range(NPAIR):
                per_k = []
                for k in range(NK):
                    t = consts.tile([128, 128], F16, tag=f"w1_{p}_{k}")
                    nc.sync.dma_start(out=t, in_=w1_t.ap()[p, k])
                    per_k.append(t)
                w1sb.append(per_k)
            w2sb = []
            for p in range(NPAIR):
                t = consts.tile([128, 32], F16, tag=f"w2_{p}")
                nc.sync.dma_start(out=t, in_=w2_t.ap()[p])
                w2sb.append(t)
            b1sb = consts.tile([128, NWP, NPAIR], F32, tag="b1")
            nc.sync.dma_start(out=b1sb, in_=b1_t.ap().rearrange("n p d -> d n p"))
            b2sb = consts.tile([96, 1], F32, tag="b2")
            nc.sync.dma_start(out=b2sb, in_=b2_t.ap())
            ersb = consts.tile([96, 1], F32, tag="er")
            nc.sync.dma_start(out=ersb, in_=er_t.ap())
            selsb = consts.tile([96, NWP, 20], F16, tag="selm")
            nc.sync.dma_start(out=selsb, in_=sel_t.ap().rearrange("n r m -> r n m"))
            ident = consts.tile([32, 32], F32, tag="ident")
            make_identity(nc, ident)

            for _ in range(reps):
                for cb in range(NCB):
                    # broadcast measurements of this column block to 96 rows
                    meas_b = wk.tile([96, CBS], F16, tag="meas_b")
                    msrc = meas_t.ap()[cb * CBS:(cb + 1) * CBS]
                    nc.sync.dma_start(
                        out=meas_b,
                        in_=bass.AP(tensor=msrc.tensor, offset=msrc.offset,
                                    ap=[[0, 96]] + msrc.ap),
                    )
                    mask = wk.tile([96, CBS], F16, tag="mask")
                    nc.vector.tensor_scalar(
                        out=mask, in0=meas_b, scalar1=ersb, scalar2=None,
                        op0=mybir.AluOpType.is_equal,
                    )

                    o20 = ps20.tile([20, CBS], F32, tag="o20")
                    for n in range(NWP):
                        row0 = cb * CBS
                        xT = []
                        for k in range(NK):
                            t = xin.tile([128, CBS], F16, tag=f"xT{k}")
                            nc.sync.dma_start_transpose(
                                out=t,
                                in_=xap[row0:row0 + CBS, n, k * 128:(k + 1) * 128],
                            )
                            xT.append(t)
                        hps = []
                        for p in range(NPAIR):
                            ps = psH.tile([128, CBS], F32, tag="hps")
                            for k in range(NK):
                                nc.tensor.matmul(ps, w1sb[p][k], xT[k],
                                                 start=(k == 0), stop=(k == NK - 1))
                            hps.append(ps)
                        hsb = []
                        for p in range(NPAIR):
                            t = hpool.tile([128, CBS], F16, tag=f"hsb{p}")
                            bias = b1sb[:, n, p:p + 1]
                            if p == 0:
                                nc.scalar.activation(
                                    out=t, in_=hps[p],
                                    func=mybir.ActivationFunctionType.Relu,
                                    bias=bias,
                                )
                            else:
                                nc.vector.tensor_scalar(
                                    out=t, in0=hps[p], scalar1=bias, scalar2=0.0,
                                    op0=mybir.AluOpType.add,
                                    op1=mybir.AluOpType.max,
                                )
                            hsb.append(t)

                        wp96 = ps96.tile([96, CBS], F32, tag="wp96")
                        for p in range(NPAIR):
                            nc.tensor.matmul(wp96[32 * p:32 * p + 32, :],
                                             w2sb[p], hsb[p],
                                             tile_position=(0, 32 * p))
                        wpb2 = wk.tile([96, CBS], F16, tag="wpb2")
                        nc.scalar.add(wpb2, wp96, add=b2sb)
                        masked = wk.tile([96, CBS], F16, tag="masked")
                        nc.vector.tensor_tensor(
                            out=masked, in0=wpb2, in1=mask,
                            op=mybir.AluOpType.mult,
                        )
                        nc.tensor.matmul(o20, selsb[:, n, :], masked,
                                         start=(n == 0), stop=(n == NWP - 1))

                    o20sb = opool.tile([20, CBS], F32, tag="o20sb")
                    nc.vector.tensor_copy(o20sb, o20)
                    for j in range(CBS // 128):
                        tp = pstp.tile([128, 20], F32, tag="tp")
                        nc.tensor.transpose(tp, o20sb[:, 128 * j:128 * (j + 1)],
                                            ident[:20, :20])
                        osb = opool.tile([128, 20], F32, tag="osb")
                        nc.vector.tensor_copy(osb, tp)
                        r0 = cb * CBS + 128 * j
                        nc.sync.dma_start(out=outap[r0:r0 + 128, :], in_=osb)

    nc.compile()
    return nc


def _prep_inputs(x, measurements, rank_embed, W1, b1, W2, b2):
    """Host-side packing of weights/constants + per-core shards."""
    x = np.asarray(x, dtype=np.float32)
    measurements = np.asarray(measurements)
    rank_embed = np.asarray(rank_embed, dtype=np.float32)
    W1 = np.asarray(W1, dtype=np.float32)
    b1 = np.asarray(b1, dtype=np.float32)
    W2 = np.asarray(W2, dtype=np.float32)
    b2 = np.asarray(b2, dtype=np.float32)

    x16 = x.astype(np.float16)                       # [BS, NWP, D]
    measf = measurements[:, 0].astype(np.float16)    # [BS]

    # b1_eff[n, e, :] = b1[e] + rank_embed[0, n, :] @ W1[e]
    b1_eff = np.einsum("nd,edh->neh", rank_embed[0], W1) + b1[None]  # [10,6,64]
    b1e = np.zeros((NWP, NPAIR, 128), np.float32)
    w1p = np.zeros((NPAIR, NK, 128, 128), np.float16)
    w2b = np.zeros((NPAIR, 128, 32), np.float16)
    b2r = np.zeros((96, 1), np.float32)
    erow = np.full((96, 1), -1.0, np.float32)
    for p in range(NPAIR):
        for i in range(2):
            e = 2 * p + i
            b1e[:, p, 64 * i:64 * i + 64] = b1_eff[:, e, :]
            for k in range(NK):
                w1p[p, k, :, 64 * i:64 * i + 64] = W1[e, 128 * k:128 * (k + 1), :]
            w2b[p, 64 * i:64 * i + 64, 2 * i:2 * i + 2] = W2[e]
            for o in range(OUT):
                r = 32 * p + 2 * i + o
                b2r[r, 0] = b2[e, o]
                erow[r, 0] = float(e)
    selm = np.zeros((NWP, 96, 20), np.float16)
    for n in range(NWP):
        for p in range(NPAIR):
            for i in range(2):
                for o in range(OUT):
                    r = 32 * p + 2 * i + o
                    for q in range(n, NWP):
                        selm[n, r, 2 * q + o] = 1.0

    shared = {"w1p": w1p, "w2b": w2b, "b1e": b1e, "b2r": b2r,
              "erow": erow, "selm": selm}
    in_maps = []
    for c in range(N_CORES):
        s = slice(c * BSH, (c + 1) * BSH)
        in_maps.append({"x16": np.ascontiguousarray(x16[s]),
                        "measf": np.ascontiguousarray(measf[s]), **shared})
    return in_maps


_NC_CACHE = {}


def kernel(x, measurements, rank_embed, W1, b1, W2, b2):
    if "nc" not in _NC_CACHE:
        _NC_CACHE["nc"] = _build_program()
    nc = _NC_CACHE["nc"]
    in_maps = _prep_inputs(x, measurements, rank_embed, W1, b1, W2, b2)
    res = run_bass_kernel_spmd(nc, in_maps, list(range(N_CORES)))
    return np.concatenate([res.results[c]["out"] for c in range(N_CORES)], axis=0)
